# revision 3
# baseline (speedup 1.0000x reference)
"""CompositionalAttention TRN2 kernel — wire-optimized SPMD version.

Full (unsharded) inputs in, full output out.  8 NeuronCores: data-parallel
over batch (4 cores per batch element) x parallel over query/key rows
(512 rows per core).  The end-to-end wall time of a call is dominated by
the host<->device tunnel (~45 MB/s), so every byte is shipped exactly once
in fp16 and reconstructed on-device with collectives:

  - x is shipped row-sharded (each core gets its own 512-row slice, fp16).
  - every weight is shipped 1/8-row-sharded (fp16) and AllGathered across
    all 8 cores on-device.
  - each core computes the s/r projections for its own 512 keys only, then
    AllGathers sk (fp16) and rv (f32) within its 4-core batch group to get
    the full 2048-key tensors.
  - the output slice is returned fp16 and upcast on host.

Math (per batch b, search head s, query row i) — unchanged from baseline:
  sq = (x @ Wsq) * sc ; sk = x @ Wsk          (per head, d=64)
  P  = softmax_j(sq_i . sk_j)                 (n x n attention)
  U_r = P @ rv_r                              (rv = x @ Wrv, r=0,1)
  retrieved_r = U_r / l,  l = sum_j expP
  sim_r = rq . (retrieved_r @ Wrk) = rowdot(U_r, rq @ Wrk^T) / l
  attn = softmax_r(sim)  ==  sigmoid(sim_0 - sim_1) for r=2 (exact)
  out_s = attn*retrieved_0 + (1-attn)*retrieved_1
  out = concat_s(out_s) @ Wout

Host folds: scale into Wsq; Wrk into Wrq (rqW = x @ (sc * Wrq_s @ Wrk^T));
mask into an additive exp bias.  exp is computed without max-subtraction
(sim ~ N(0,1), max |sim| << 80, so fp32/bf16 exp is safe).
"""

import sys

sys.path.insert(0, "/opt/trn_rl_repo")

import numpy as np

B, N, DIM, S, R, DH = 2, 2048, 1024, 8, 2, 64
SD, RD = S * DH, R * DH  # 512, 128
NCORES = 8
GROUP = 4  # cores per batch element
NSLICE = N // GROUP  # 512 rows per core
SCALE = DH**-0.5
KT = DIM // 128  # 8 contraction tiles
JT = N // 128  # 16 key tiles
ICN = NSLICE // 128  # 4 query chunks
PAIRS = S // 2
WOS = SD // NCORES  # 64 wout shard rows

_cache = {}


def _build_program():
    import concourse.bass as bass
    import concourse.tile as tile
    from concourse import bacc, mybir
    from concourse.masks import make_identity

    f32 = mybir.dt.float32
    f16 = mybir.dt.float16
    bf16 = mybir.dt.bfloat16
    Exp = mybir.ActivationFunctionType.Exp
    Sigmoid = mybir.ActivationFunctionType.Sigmoid
    add = mybir.AluOpType.add
    bypass = mybir.AluOpType.bypass

    nc = bacc.Bacc(
        "TRN2", target_bir_lowering=False, debug=False, num_devices=NCORES
    )

    xsTd = nc.dram_tensor("xsT", [DIM, NSLICE], f16, kind="ExternalInput").ap()
    mbd = nc.dram_tensor("mb", [N], f32, kind="ExternalInput").ap()
    wsqd = nc.dram_tensor("wsq", [128, SD], f16, kind="ExternalInput").ap()
    wskd = nc.dram_tensor("wsk", [128, SD], f16, kind="ExternalInput").ap()
    wrqd = nc.dram_tensor("wrq", [128, SD], f16, kind="ExternalInput").ap()
    wrvd = nc.dram_tensor("wrv", [128, RD], f16, kind="ExternalInput").ap()
    woutd = nc.dram_tensor("wout", [WOS, DIM], f16, kind="ExternalInput").ap()
    outd = nc.dram_tensor("out", [NSLICE, DIM], f16, kind="ExternalOutput").ap()

    ALL8 = [list(range(NCORES))]
    GROUPS4 = [[0, 1, 2, 3], [4, 5, 6, 7]]

    with tile.TileContext(nc) as tc:
        with (
            tc.tile_pool(name="sk", bufs=4) as skp,
            tc.tile_pool(name="sq", bufs=4) as sqp,
            tc.tile_pool(name="rqw", bufs=4) as rqwp,
            tc.tile_pool(name="rvaug", bufs=JT) as rvap,
            tc.tile_pool(name="consts", bufs=4) as constp,
            tc.tile_pool(name="outcat", bufs=4) as outcatp,
            tc.tile_pool(name="wsb", bufs=3 * KT + KT + 4) as wsbp,
            tc.tile_pool(name="dram", bufs=16, space="DRAM") as dramp,
            tc.tile_pool(name="psA", bufs=2, space="PSUM") as psA,
        ):
            # ---- weight shard DMAs + 8-wide AllGathers (start immediately) ----
            wsq_in = dramp.tile([128, SD], f16, tag="wsq_in", name="wsq_in")
            wsk_in = dramp.tile([128, SD], f16, tag="wsk_in", name="wsk_in")
            wrq_in = dramp.tile([128, SD], f16, tag="wrq_in", name="wrq_in")
            wrv_in = dramp.tile([128, RD], f16, tag="wrv_in", name="wrv_in")
            wout_in = dramp.tile([WOS, DIM], f16, tag="wout_in", name="wout_in")
            wsq_all = dramp.tile([DIM, SD], f16, tag="wsq_all", name="wsq_all")
            wsk_all = dramp.tile([DIM, SD], f16, tag="wsk_all", name="wsk_all")
            wrq_all = dramp.tile([DIM, SD], f16, tag="wrq_all", name="wrq_all")
            wrv_all = dramp.tile([DIM, RD], f16, tag="wrv_all", name="wrv_all")
            wout_all = dramp.tile([SD, DIM], f16, tag="wout_all", name="wout_all")
            for bi, ba, ext in (
                (wsk_in, wsk_all, wskd),
                (wsq_in, wsq_all, wsqd),
                (wrv_in, wrv_all, wrvd),
                (wrq_in, wrq_all, wrqd),
                (wout_in, wout_all, woutd),
            ):
                nc.gpsimd.dma_start(bi[:], ext)
                nc.gpsimd.collective_compute(
                    "AllGather",
                    bypass,
                    replica_groups=ALL8,
                    ins=[bi.opt()],
                    outs=[ba.opt()],
                )

            # bounce buffers for the 4-wide sk/rv gathers
            skg_in = dramp.tile([SD, NSLICE], f16, tag="skg_in", name="skg_in")
            skg_all = dramp.tile(
                [GROUP * SD, NSLICE], f16, tag="skg_all", name="skg_all"
            )
            rvg_in = dramp.tile([RD, NSLICE], f32, tag="rvg_in", name="rvg_in")
            rvg_all = dramp.tile(
                [GROUP * RD, NSLICE], f32, tag="rvg_all", name="rvg_all"
            )

            # ---- constants ----
            mb = constp.tile([128, JT], f32, tag="mb", name="mb")
            nc.sync.dma_start(mb[:], mbd.rearrange("(t p) -> p t", p=128))
            identity = constp.tile([128, 128], f32, tag="ident", name="ident")
            make_identity(nc, identity[:])

            skT = [skp.tile([128, N], f16, tag="skT", name="skT") for _ in range(4)]
            sqT = [sqp.tile([128, NSLICE], f16, tag="sqT", name="sqT") for _ in range(4)]
            rqW = [rqwp.tile([128, SD], f32, tag="rqW", name="rqW") for _ in range(4)]
            rvaug = [rvap.tile([128, 132], bf16, tag="rvaug", name="rvaug") for _ in range(JT)]

            # ============ Phase 1: projections + gathers ============
            with (
                tc.tile_pool(name="xh", bufs=KT) as xhp,
                tc.tile_pool(name="stg", bufs=6) as stgp,
                tc.tile_pool(name="rvbf", bufs=1) as rvbfp,
            ):
                # own x slice, fp16 [128 k, 512 rows] per contraction tile
                xh = []
                for kt in range(KT):
                    t = xhp.tile([128, NSLICE], f16, tag="xh", name="xh")
                    nc.sync.dma_start(t[:], xsTd[kt * 128 : (kt + 1) * 128, :])
                    xh.append(t)

                # full fp16 weights into SBUF (after gathers)
                wskh, wsqh, wrqh, wrvh = [], [], [], []
                for dst, src, cols in (
                    (wskh, wsk_all, SD),
                    (wrvh, wrv_all, RD),
                    (wsqh, wsq_all, SD),
                    (wrqh, wrq_all, SD),
                ):
                    for kt in range(KT):
                        t = wsbp.tile([128, cols], f16, tag=f"w{cols}", name="wh")
                        nc.sync.dma_start(t[:], src[kt * 128 : (kt + 1) * 128, :])
                        dst.append(t)

                # sk slice: [512 d, 512 j_own] -> fp16 -> DRAM -> group gather
                for dt in range(4):
                    ps = psA.tile([128, 512], f32, tag="psA", name="psA")
                    for kt in range(KT):
                        nc.tensor.matmul(
                            ps[:],
                            wskh[kt][:, dt * 128 : (dt + 1) * 128],
                            xh[kt][:],
                            start=(kt == 0),
                            stop=(kt == KT - 1),
                        )
                    st = stgp.tile([128, NSLICE], f16, tag="st16", name="st16")
                    nc.vector.tensor_copy(st[:], ps[:])
                    nc.gpsimd.dma_start(
                        skg_in[dt * 128 : (dt + 1) * 128, :], st[:]
                    )

                # rv slice: [128 rd, 512 j_own] -> f32 -> DRAM -> group gather
                ps = psA.tile([128, 512], f32, tag="psA", name="psA")
                for kt in range(KT):
                    nc.tensor.matmul(
                        ps[:],
                        wrvh[kt][:],
                        xh[kt][:],
                        start=(kt == 0),
                        stop=(kt == KT - 1),
                    )
                rvst = stgp.tile([128, NSLICE], f32, tag="st32", name="st32")
                nc.vector.tensor_copy(rvst[:], ps[:])
                nc.gpsimd.dma_start(rvg_in[:, :], rvst[:])

                nc.gpsimd.collective_compute(
                    "AllGather",
                    bypass,
                    replica_groups=GROUPS4,
                    ins=[skg_in.opt()],
                    outs=[skg_all.opt()],
                )
                nc.gpsimd.collective_compute(
                    "AllGather",
                    bypass,
                    replica_groups=GROUPS4,
                    ins=[rvg_in.opt()],
                    outs=[rvg_all.opt()],
                )

                # scatter gathered sk into skT tiles [128 d, 2048 j]
                for dt in range(4):
                    for g in range(GROUP):
                        nc.sync.dma_start(
                            skT[dt][:, g * NSLICE : (g + 1) * NSLICE],
                            skg_all[
                                g * SD + dt * 128 : g * SD + (dt + 1) * 128, :
                            ],
                        )

                # sqT[dt]: [128 d, 512 i_own] (scale pre-folded into Wsq)
                for dt in range(4):
                    ps = psA.tile([128, 512], f32, tag="psA", name="psA")
                    for kt in range(KT):
                        nc.tensor.matmul(
                            ps[:],
                            wsqh[kt][:, dt * 128 : (dt + 1) * 128],
                            xh[kt][:],
                            start=(kt == 0),
                            stop=(kt == KT - 1),
                        )
                    nc.vector.tensor_copy(sqT[dt][:], ps[:])

                # rqW[ic]: row-land [128 i, 512 sd] = x_i @ (sc*Wrq_s@Wrk^T)
                for ic in range(ICN):
                    ps = psA.tile([128, 512], f32, tag="psA", name="psA")
                    for kt in range(KT):
                        nc.tensor.matmul(
                            ps[:],
                            xh[kt][:, ic * 128 : (ic + 1) * 128],
                            wrqh[kt][:],
                            start=(kt == 0),
                            stop=(kt == KT - 1),
                        )
                    nc.vector.tensor_copy(rqW[ic][:], ps[:])

                # rv full [128 rd, 2048 j] -> transpose to rv_aug [j, 132] bf16
                rvbf = rvbfp.tile([128, N], f32, tag="rvbf", name="rvbf")
                for g in range(GROUP):
                    nc.sync.dma_start(
                        rvbf[:, g * NSLICE : (g + 1) * NSLICE],
                        rvg_all[g * RD : (g + 1) * RD, :],
                    )
                for jt in range(JT):
                    nc.gpsimd.memset(rvaug[jt][:], 1.0)
                for g in range(4):
                    ps = psA.tile([128, 512], f32, tag="psA", name="psA")
                    for k in range(4):
                        jt = g * 4 + k
                        nc.tensor.transpose(
                            ps[:, k * 128 : (k + 1) * 128],
                            rvbf[:, jt * 128 : (jt + 1) * 128],
                            identity[:],
                        )
                    for k in range(4):
                        jt = g * 4 + k
                        nc.vector.tensor_copy(
                            rvaug[jt][:, 0:128], ps[:, k * 128 : (k + 1) * 128]
                        )

            # ============ Phase 2: attention + retrieval ============
            wouth = []
            for sc in range(4):
                t = wsbp.tile([128, DIM], f16, tag="wout_sb", name="wout_sb")
                nc.sync.dma_start(t[:], wout_all[sc * 128 : (sc + 1) * 128, :])
                wouth.append(t)

            outcat = [outcatp.tile([128, SD], f32, tag="outcat", name="outcat") for _ in range(4)]

            with (
                tc.tile_pool(name="expp", bufs=36) as expp,
                tc.tile_pool(name="small", bufs=16) as smallp,
                tc.tile_pool(name="scr", bufs=4) as scrp,
                tc.tile_pool(name="psQK", bufs=2, space="PSUM") as psQK,
                tc.tile_pool(name="psU", bufs=4, space="PSUM") as psU,
            ):
                for p in range(PAIRS):
                    expP = [[None] * JT, [None] * JT]
                    for jt in range(JT):
                        for h in range(2):
                            qk = psQK.tile([128, 512], f32, tag="qk", name="qk")
                            lo, hi = h * 64, (h + 1) * 64
                            nc.tensor.matmul(
                                qk[:],
                                skT[p][lo:hi, jt * 128 : (jt + 1) * 128],
                                sqT[p][lo:hi, :],
                                start=True,
                                stop=True,
                            )
                            e = expp.tile([128, 512], bf16, tag="expP", name="expP")
                            nc.scalar.activation(
                                e[:], qk[:], Exp, bias=mb[:, jt : jt + 1], scale=1.0
                            )
                            expP[h][jt] = e
                    for h in range(2):
                        s = 2 * p + h
                        U = [psU.tile([128, 129], f32, tag="U", name="U") for _ in range(ICN)]
                        for jt in range(JT):
                            for ic in range(ICN):
                                nc.tensor.matmul(
                                    U[ic][:],
                                    expP[h][jt][:, ic * 128 : (ic + 1) * 128],
                                    rvaug[jt][:, 0:129],
                                    start=(jt == 0),
                                    stop=(jt == JT - 1),
                                )
                        # retrieval stage (row-land, all per-partition scalars)
                        Usb = []
                        for ic in range(ICN):
                            u = scrp.tile([128, 129], f32, tag="Usb", name="Usb")
                            nc.vector.tensor_copy(u[:], U[ic][:, 0:129])
                            Usb.append(u)
                        Bt = smallp.tile([128, 8], f32, tag="Bt", name="Bt")
                        for ic in range(ICN):
                            for r in range(R):
                                prod = scrp.tile([128, 64], f32, tag="prod", name="prod")
                                nc.vector.tensor_mul(
                                    prod[:],
                                    Usb[ic][:, r * 64 : (r + 1) * 64],
                                    rqW[ic][:, s * 64 : (s + 1) * 64],
                                )
                                nc.vector.tensor_reduce(
                                    Bt[:, r * 4 + ic : r * 4 + ic + 1],
                                    prod[:],
                                    axis=mybir.AxisListType.X,
                                    op=add,
                                )
                        lcol = smallp.tile([128, 4], f32, tag="lcol", name="lcol")
                        for ic in range(ICN):
                            nc.vector.tensor_copy(
                                lcol[:, ic : ic + 1], Usb[ic][:, 128:129]
                            )
                        linv = smallp.tile([128, 4], f32, tag="linv", name="linv")
                        nc.vector.reciprocal(linv[:], lcol[:])
                        dd = smallp.tile([128, 4], f32, tag="dd", name="dd")
                        nc.vector.tensor_sub(dd[:], Bt[:, 0:4], Bt[:, 4:8])
                        nc.vector.tensor_mul(dd[:], dd[:], linv[:])
                        g = smallp.tile([128, 4], f32, tag="g", name="g")
                        nc.scalar.activation(g[:], dd[:], Sigmoid)
                        w0 = smallp.tile([128, 4], f32, tag="w0", name="w0")
                        nc.vector.tensor_mul(w0[:], g[:], linv[:])
                        w1 = smallp.tile([128, 4], f32, tag="w1", name="w1")
                        nc.vector.tensor_sub(w1[:], linv[:], w0[:])
                        for ic in range(ICN):
                            v0 = scrp.tile([128, 64], f32, tag="v0", name="v0")
                            nc.vector.tensor_scalar_mul(
                                v0[:], Usb[ic][:, 0:64], w0[:, ic : ic + 1]
                            )
                            v1 = scrp.tile([128, 64], f32, tag="v1", name="v1")
                            nc.vector.tensor_scalar_mul(
                                v1[:], Usb[ic][:, 64:128], w1[:, ic : ic + 1]
                            )
                            nc.vector.tensor_add(
                                outcat[ic][:, s * 64 : (s + 1) * 64], v0[:], v1[:]
                            )

            # ============ Phase 3: output projection ============
            with (
                tc.tile_pool(name="octT", bufs=4) as octTp,
                tc.tile_pool(name="osb", bufs=3) as osbp,
                tc.tile_pool(name="psT", bufs=2, space="PSUM") as psT,
            ):
                octT = [
                    octTp.tile([128, NSLICE], f16, tag="octT", name="octT") for _ in range(4)
                ]
                for ic in range(ICN):
                    for sc in range(4):
                        tp = psT.tile([128, 128], f32, tag="tp", name="tp")
                        nc.tensor.transpose(
                            tp[:],
                            outcat[ic][:, sc * 128 : (sc + 1) * 128],
                            identity[:],
                        )
                        nc.vector.tensor_copy(
                            octT[sc][:, ic * 128 : (ic + 1) * 128], tp[:]
                        )
                for ic in range(ICN):
                    ot = osbp.tile([128, DIM], f16, tag="osb", name="osb")
                    for half in range(2):
                        ps = psA.tile([128, 512], f32, tag="psA", name="psA")
                        for sc in range(4):
                            nc.tensor.matmul(
                                ps[:],
                                octT[sc][:, ic * 128 : (ic + 1) * 128],
                                wouth[sc][:, half * 512 : (half + 1) * 512],
                                start=(sc == 0),
                                stop=(sc == 3),
                            )
                        nc.vector.tensor_copy(
                            ot[:, half * 512 : (half + 1) * 512], ps[:]
                        )
                    nc.sync.dma_start(
                        outd[ic * 128 : (ic + 1) * 128, :], ot[:]
                    )

    nc.compile()
    return nc


def _prep_in_maps(x, mask, Wsq, Wsk, Wrv, Wrq, Wrk, Wout):
    x = np.asarray(x, dtype=np.float32)
    mask = np.asarray(mask)
    Wsq = np.asarray(Wsq, dtype=np.float32)
    Wsk = np.asarray(Wsk, dtype=np.float32)
    Wrv = np.asarray(Wrv, dtype=np.float32)
    Wrq = np.asarray(Wrq, dtype=np.float32)
    Wrk = np.asarray(Wrk, dtype=np.float32)
    Wout = np.asarray(Wout, dtype=np.float32)

    wsq_eff = (Wsq * np.float32(SCALE)).astype(np.float16)
    # rqW = x @ wrq_eff where wrq_eff per head s: SCALE * Wrq_s @ Wrk^T
    wrq_eff = np.empty_like(Wrq)
    for s in range(S):
        wrq_eff[:, s * DH : (s + 1) * DH] = (
            Wrq[:, s * DH : (s + 1) * DH] @ Wrk.T
        ) * np.float32(SCALE)
    wrq_eff = wrq_eff.astype(np.float16)
    wsk16 = Wsk.astype(np.float16)
    wrv16 = Wrv.astype(np.float16)
    wout16 = Wout.astype(np.float16)
    mb = np.where(mask, np.float32(0.0), np.float32(-1e30)).astype(np.float32)

    xT16 = [np.ascontiguousarray(x[b].T.astype(np.float16)) for b in range(B)]

    in_maps = []
    for c in range(NCORES):
        bc, gr = c // GROUP, c % GROUP
        in_maps.append(
            {
                "xsT": np.ascontiguousarray(
                    xT16[bc][:, gr * NSLICE : (gr + 1) * NSLICE]
                ),
                "mb": mb[bc],
                "wsq": wsq_eff[c * 128 : (c + 1) * 128],
                "wsk": wsk16[c * 128 : (c + 1) * 128],
                "wrq": wrq_eff[c * 128 : (c + 1) * 128],
                "wrv": wrv16[c * 128 : (c + 1) * 128],
                "wout": wout16[c * WOS : (c + 1) * WOS],
            }
        )
    return in_maps


def _get_nc():
    if "nc" not in _cache:
        _cache["nc"] = _build_program()
    return _cache["nc"]


def _get_exec():
    """Compile the program and build a cached sharded executor.

    Differences from bass2jax.run_bass_via_pjrt: the zero-filled output
    buffers are created on device ONCE and reused (not donated), instead of
    being re-uploaded through the tunnel on every call.  The kernel writes
    every element of its output, so the initial contents never matter.
    """
    if "exec" in _cache:
        return _cache["exec"]

    import jax
    from jax.experimental.shard_map import shard_map
    from jax.sharding import Mesh, NamedSharding, PartitionSpec
    from concourse import mybir
    from concourse.bass2jax import (
        _bass_exec_p,
        install_neuronx_cc_hook,
        partition_id_tensor,
    )

    nc = _get_nc()
    install_neuronx_cc_hook()
    assert nc.dbg_addr is None

    pname = nc.partition_id_tensor.name if nc.partition_id_tensor else None
    in_names, out_names, out_avals = [], [], []
    for alloc in nc.m.functions[0].allocations:
        if not isinstance(alloc, mybir.MemoryLocationSet):
            continue
        name = alloc.memorylocations[0].name
        if alloc.kind == "ExternalInput":
            if name != pname:
                in_names.append(name)
        elif alloc.kind == "ExternalOutput":
            out_names.append(name)
            out_avals.append(
                jax.core.ShapedArray(
                    tuple(alloc.tensor_shape), mybir.dt.np(alloc.dtype)
                )
            )
    n_params = len(in_names)
    all_names = list(in_names) + list(out_names)
    if pname is not None:
        all_names.append(pname)

    def _body(*args):
        operands = list(args)
        if pname is not None:
            operands.append(partition_id_tensor())
        outs = _bass_exec_p.bind(
            *operands,
            out_avals=tuple(out_avals),
            in_names=tuple(all_names),
            out_names=tuple(out_names),
            lowering_input_output_aliases=(),
            sim_require_finite=True,
            sim_require_nnan=True,
            nc=nc,
        )
        return tuple(outs)

    devices = jax.devices()[:NCORES]
    mesh = Mesh(np.asarray(devices), ("core",))
    in_specs = (PartitionSpec("core"),) * (n_params + len(out_names))
    out_specs = (PartitionSpec("core"),) * len(out_names)
    fn = jax.jit(
        shard_map(
            _body, mesh=mesh, in_specs=in_specs, out_specs=out_specs,
            check_rep=False,
        ),
        keep_unused=True,
    )
    sh = NamedSharding(mesh, PartitionSpec("core"))
    zeros = [
        jax.device_put(
            np.zeros((NCORES * a.shape[0], *a.shape[1:]), a.dtype), sh
        )
        for a in out_avals
    ]
    _cache["exec"] = (fn, in_names, out_names, out_avals, zeros)
    return _cache["exec"]


def _fetch_sharded(arr):
    """Fetch a sharded jax array device->host with one thread per shard."""
    if "pool" not in _cache:
        from concurrent.futures import ThreadPoolExecutor

        _cache["pool"] = ThreadPoolExecutor(NCORES)
    shards = sorted(
        arr.addressable_shards, key=lambda s: s.index[0].start or 0
    )
    datas = list(_cache["pool"].map(lambda s: np.asarray(s.data), shards))
    return np.concatenate(datas, axis=0)


def _run(in_maps):
    fn, in_names, out_names, out_avals, zeros = _get_exec()
    concat_in = [
        np.concatenate(
            [np.asarray(in_maps[c][name]) for c in range(NCORES)], axis=0
        )
        for name in in_names
    ]
    out_arrs = fn(*concat_in, *zeros)
    fetched = [_fetch_sharded(a) for a in out_arrs]
    return [
        {
            name: fetched[i].reshape(NCORES, *out_avals[i].shape)[c]
            for i, name in enumerate(out_names)
        }
        for c in range(NCORES)
    ]


def kernel(**inputs):
    in_maps = _prep_in_maps(
        inputs["x"],
        inputs["mask"],
        inputs["Wsq"],
        inputs["Wsk"],
        inputs["Wrv"],
        inputs["Wrq"],
        inputs["Wrk"],
        inputs["Wout"],
    )
    results = _run(in_maps)
    out = np.empty((B, N, DIM), dtype=np.float32)
    for c in range(NCORES):
        bc, gr = c // GROUP, c % GROUP
        out[bc, gr * NSLICE : (gr + 1) * NSLICE, :] = results[c]["out"].astype(
            np.float32
        )
    return out
